# revision 1
# baseline (speedup 1.0000x reference)
"""Tensor-parallel MHA kernel v2 for 8 Trainium2 NeuronCores.

vs baseline:
- ACT runs ONLY the exp stream + 6 batched rms-rsqrt chains (Ln+Exp on
  partition-packed ssq tiles read straight from PSUM, ~1.4us each).
- squares on DVE; rinv broadcasts on gpsimd from gathered rows.
- ssq matmuls pack 2-4 granules into one PSUM tile via tile_position
  column offsets {0,32,64,96}; the ACT chain reads it directly.
- y-normalization: DVE reciprocal of the psum den row + gpsimd bcast.
- outproj emitted as per-dtile micro-fillers; PSUM hand-allocated:
  2 QK slots (4 banks) + yA/yB (2) + psA proj (1) + psB misc (1).

Chains produce rinv/8 = rsqrt(ssq) (ssq = 64*ms); with the 1/sqrt(hd)
attention scale the exp instruction uses scale = 64/8 = 8.0.
"""

import sys

if "/opt/trn_rl_repo" not in sys.path:
    sys.path.insert(0, "/opt/trn_rl_repo")

import numpy as np
import ml_dtypes

BF16 = ml_dtypes.bfloat16

B, T, C = 2, 2048, 1024
H, HD = 16, 64
BT = B * T
NCORES = 8
DPC = C // NCORES
NKT = T // 128
NCI = C // 128
EXPSCALE = 8.0
EPS = float(np.finfo(np.float32).eps)
MLN8 = -float(np.log(8.0))
JQ = 512
NJQ = T // JQ
GR = 512
NG = T // GR
NEG = -30000.0

_CACHE = {}

# chain groups: members (tensor, batch, granule); member j's ssq lands in
# COLUMNS [j*GR, (j+1)*GR) of rows {0,1} of the shared collect tile.
CHAIN_GROUPS = [
    [("q", 0, 0), ("k", 0, 0)],
    [("k", 0, 1), ("k", 0, 2)],
    [("k", 0, 3), ("q", 0, 1)],
    [("q", 0, 2), ("q", 0, 3)],
    [("k", 1, 0), ("k", 1, 1)],
    [("k", 1, 2), ("k", 1, 3)],
    [("q", 1, 0), ("q", 1, 1)],
    [("q", 1, 2), ("q", 1, 3)],
]
MAXW = max(len(g) for g in CHAIN_GROUPS) * GR


def _build_bass():
    import os
    import concourse.bass as bass
    from concourse import bacc, mybir, tile
    from concourse.masks import make_identity
    from contextlib import ExitStack

    dt = mybir.dt
    AF = mybir.ActivationFunctionType
    ts = bass.ts

    from concourse import bacc as _bacc_mod, hw_specs as _hw

    _orig_tables = _hw.get_activation_tables

    def _only_nl_exp(arch):
        t = _orig_tables(arch)
        return {
            name: (fns if name == "natural_log_exp_and_others" else set())
            for name, fns in t.items()
        }

    _bacc_mod.get_activation_tables = _only_nl_exp

    nc = bacc.Bacc("TRN2", target_bir_lowering=False, debug=False)

    xt_ext = nc.dram_tensor("xt", [128, NCI * BT], dt.bfloat16, kind="ExternalInput")
    wq_ext = nc.dram_tensor("wq", [128, NCI * DPC], dt.bfloat16, kind="ExternalInput")
    wk_ext = nc.dram_tensor("wk", [128, NCI * DPC], dt.bfloat16, kind="ExternalInput")
    wv_ext = nc.dram_tensor("wv", [128, NCI * DPC], dt.bfloat16, kind="ExternalInput")
    wo_ext = nc.dram_tensor("wo", [DPC, C], dt.bfloat16, kind="ExternalInput")
    bias_ext = nc.dram_tensor("bias", [128, B * NKT], dt.float32, kind="ExternalInput")
    out_ext = nc.dram_tensor("out", [128, NCI * BT], dt.bfloat16, kind="ExternalOutput")

    with ExitStack() as ctx:
        tc = ctx.enter_context(tile.TileContext(nc))
        singles = ctx.enter_context(tc.tile_pool(name="singles", bufs=1))
        rawp = ctx.enter_context(tc.tile_pool(name="rawp", bufs=8))
        sqp = ctx.enter_context(tc.tile_pool(name="sqp", bufs=2))
        sep = ctx.enter_context(tc.tile_pool(name="sep", bufs=4))
        obp = ctx.enter_context(tc.tile_pool(name="obp", bufs=3))
        stgp = ctx.enter_context(tc.tile_pool(name="stgp", bufs=3))
        rbcp = ctx.enter_context(tc.tile_pool(name="rbcp", bufs=3))
        rbtp = ctx.enter_context(tc.tile_pool(name="rbtp", bufs=3))
        ynbp = ctx.enter_context(tc.tile_pool(name="ynbp", bufs=2))
        chnp = ctx.enter_context(tc.tile_pool(name="chnp", bufs=2))
        psing = ctx.enter_context(tc.tile_pool(name="psing", bufs=1, space="PSUM"))
        ycp = ctx.enter_context(tc.tile_pool(name="ycp", bufs=2))

        # ---- persistent SBUF ----
        xt_sb = singles.tile([128, NCI * BT], dt.bfloat16)
        wq_sb = singles.tile([128, NCI * DPC], dt.bfloat16)
        wk_sb = singles.tile([128, NCI * DPC], dt.bfloat16)
        wv_sb = singles.tile([128, NCI * DPC], dt.bfloat16)
        wo_sb = singles.tile([128, C], dt.bfloat16)
        bias_sb = singles.tile([128, B * NKT], dt.float32)
        qn_sb = singles.tile([128, BT], dt.bfloat16)
        kn_sb = singles.tile([128, BT], dt.bfloat16)
        vext = singles.tile([128, 2, B, NKT, HD + 1], dt.bfloat16)
        yn_sb = singles.tile([128, BT], dt.bfloat16)
        ones2 = singles.tile([128, 2], dt.bfloat16)
        ident = singles.tile([128, 128], dt.bfloat16)
        eps_col = singles.tile([128, 1], dt.float32)
        mln8_col = singles.tile([128, 1], dt.float32)
        group_coll = {}
        group_rinv = {}

        # ---- manual PSUM (exactly 8 banks) ----
        slotA = psing.tile([128, 2 * JQ], dt.float32, name="slotA")
        slotB = psing.tile([128, 2 * JQ], dt.float32, name="slotB")
        yA_t = psing.tile([65, JQ], dt.float32, name="yA")
        yB_t = psing.tile([65, JQ], dt.float32, name="yB")
        psA = psing.tile([128, GR], dt.float32, name="psA")
        psB = psing.tile([128, GR], dt.float32, name="psB")
        psB_bf = psB[:].bitcast(dt.bfloat16)  # [128, 1024] bf16 view

        nc.sync.dma_start(out=wq_sb[:], in_=wq_ext.ap())
        nc.sync.dma_start(out=wk_sb[:], in_=wk_ext.ap())
        nc.sync.dma_start(out=wv_sb[:], in_=wv_ext.ap())
        nc.sync.dma_start(out=wo_sb[:], in_=wo_ext.ap())
        nc.sync.dma_start(out=bias_sb[:], in_=bias_ext.ap())
        NTG = BT // GR
        for tg in range(NTG):
            w = NCI * GR
            nc.sync.dma_start(
                out=xt_sb[:, tg * w : (tg + 1) * w],
                in_=xt_ext.ap()[:, tg * w : (tg + 1) * w],
            )

        nc.gpsimd.memset(eps_col[:], EPS)
        nc.gpsimd.memset(mln8_col[:], MLN8)
        nc.gpsimd.memset(ones2[:], 0.0)
        nc.gpsimd.memset(ones2[0:64, 0:1], 1.0)
        nc.gpsimd.memset(ones2[64:128, 1:2], 1.0)
        nc.gpsimd.memset(vext[:, :, :, :, HD : HD + 1], 1.0)
        make_identity(nc, ident[:])

        raw_tiles = {}
        member_pos = {}
        for gi, members in enumerate(CHAIN_GROUPS):
            for j, m in enumerate(members):
                member_pos[m] = (gi, j)
        vstate = {}

        def proj_psum(w_sb, t0, dst=None):
            if dst is None:
                dst = psA
            tg = t0 // GR
            for ci in range(NCI):
                c0 = tg * NCI * GR + ci * GR
                nc.tensor.matmul(
                    dst[:],
                    lhsT=w_sb[:, ts(ci, DPC)],
                    rhs=xt_sb[:, c0 : c0 + GR],
                    start=(ci == 0),
                    stop=(ci == NCI - 1),
                )

        def qk_part1(tensor, b, g):
            gi, j = member_pos[(tensor, b, g)]
            w_sb = wq_sb if tensor == "q" else wk_sb
            proj_psum(w_sb, b * T + g * GR)
            raw = rawp.tile([128, GR], dt.bfloat16, tag="raw")
            nc.vector.tensor_copy(raw[:], psA[:])
            raw_tiles[(tensor, b, g)] = raw
            q2 = sqp.tile([128, GR], dt.bfloat16, tag="q2")
            nc.vector.tensor_mul(q2[:], raw[:], raw[:])
            c0 = j * GR
            nc.tensor.matmul(
                psB[0:2, :],
                lhsT=ones2[:],
                rhs=q2[:],
                start=True,
                stop=True,
            )
            if gi not in group_coll:
                group_coll[gi] = chnp.tile([2, MAXW], dt.float32, tag="coll",
                                           name=f"coll_g{gi}")
            nc.vector.tensor_copy(group_coll[gi][0:2, c0 : c0 + GR], psB[0:2, :])

        def chain(gi):
            w = len(CHAIN_GROUPS[gi]) * GR
            coll = group_coll.pop(gi)
            lnt = chnp.tile([2, MAXW], dt.float32, tag="lnt", name=f"lnt_g{gi}")
            rinv = chnp.tile([2, MAXW], dt.float32, tag="rinv", name=f"rinv_g{gi}")
            group_rinv[gi] = rinv
            nc.scalar.activation(
                out=lnt[:, 0:w], in_=coll[:, 0:w], func=AF.Ln,
                bias=eps_col[0:2, :], scale=1.0 / HD,
            )
            nc.scalar.activation(
                out=rinv[:, 0:w], in_=lnt[:, 0:w], func=AF.Exp,
                bias=mln8_col[0:2, :], scale=-0.5,
            )

        def qk_finalize(tensor, b, g):
            gi, j = member_pos[(tensor, b, g)]
            c0 = j * GR
            stA = stgp.tile([1, GR], dt.float32, tag="st")
            stB = stgp.tile([1, GR], dt.float32, tag="st")
            rinv = group_rinv[gi]
            nc.sync.dma_start(out=stA[:], in_=rinv[0:1, c0 : c0 + GR])
            nc.sync.dma_start(out=stB[:], in_=rinv[1:2, c0 : c0 + GR])
            rbc = rbcp.tile([128, GR], dt.float32, tag="rbc")
            rbB = rbtp.tile([64, GR], dt.float32, tag="rbB")
            nc.gpsimd.partition_broadcast(rbc[0:64, :], stA[:])
            nc.gpsimd.partition_broadcast(rbB[:], stB[:])
            nc.sync.dma_start(out=rbc[64:128, :], in_=rbB[:])
            dst = qn_sb if tensor == "q" else kn_sb
            t0 = b * T + g * GR
            raw = raw_tiles.pop((tensor, b, g))
            nc.vector.tensor_mul(dst[:, t0 : t0 + GR], raw[:], rbc[:])

        def v_proj(b, g):
            proj_psum(wv_sb, b * T + g * GR)
            vt = rawp.tile([128, GR], dt.bfloat16, tag="raw", name="vt")
            nc.vector.tensor_copy(vt[:], psA[:])
            vstate[(b, g)] = vt

        def v_block(b, g, jblk):
            vt = vstate[(b, g)]
            pst = psB_bf[:, 0:128]
            nc.tensor.transpose(pst, vt[:, ts(jblk, 128)], ident[:])
            kt = g * (GR // 128) + jblk
            nc.vector.tensor_copy(vext[:, 0, b, kt, 0:HD], pst[:, 0:HD])
            nc.vector.tensor_copy(vext[:, 1, b, kt, 0:HD], pst[:, HD : 2 * HD])

        def outproj_dtile(b, ch, dtile):
            t0 = b * T + ch * JQ
            nc.tensor.matmul(
                psB[:],
                lhsT=wo_sb[:, ts(dtile, 128)],
                rhs=yn_sb[:, t0 : t0 + JQ],
                start=True,
                stop=True,
            )
            ob = obp.tile([128, JQ], dt.bfloat16, tag="ob")
            nc.vector.tensor_copy(ob[:], psB[:])
            dst = out_ext.ap().rearrange("p (n t) -> p n t", n=NCI)[
                :, dtile, t0 : t0 + JQ
            ]
            nc.sync.dma_start(out=dst, in_=ob[:])

        # ---- filler schedule (it -> thunks) ----
        fillers = {}

        def add(it, *ths):
            for th in ths:
                fillers.setdefault(it, []).append(th)

        def fp1(t, b, g):
            return lambda: qk_part1(t, b, g)

        def fch(gi):
            return lambda: chain(gi)

        def ffin(t, b, g):
            return lambda: qk_finalize(t, b, g)

        def fvp(b, g):
            return lambda: v_proj(b, g)

        def fvb(b, g, j):
            return lambda: v_block(b, g, j)

        def fout(b, ch, d):
            return lambda: outproj_dtile(b, ch, d)

        def fdump_xt():
            def run():
                import os
                if bool(int(os.environ.get("BASS_ATTN_DEBUG", "0"))):
                    extd = nc.dram_tensor("dbg_xt_early", [128, NCI * BT],
                                          dt.bfloat16, kind="ExternalOutput")
                    nc.sync.dma_start(out=extd.ap(), in_=xt_sb[:])
            return run

        add(0, fp1("k", 0, 1))
        add(1, fp1("k", 0, 2))
        add(2, fch(1), ffin("k", 0, 1))
        add(3, fvb(0, 2, 0), fvb(0, 2, 1))
        add(4, ffin("k", 0, 2), fvb(0, 2, 2), fvb(0, 2, 3))
        add(5, fp1("k", 0, 3))
        add(6, fp1("q", 0, 1))
        add(7, fch(2))
        add(8, ffin("k", 0, 3), fvp(0, 3))
        add(9, ffin("q", 0, 1), fvb(0, 3, 0))
        add(10, fvb(0, 3, 1), fvb(0, 3, 2))
        add(11, fvb(0, 3, 3))
        add(12, fp1("q", 0, 2))
        add(13, fp1("q", 0, 3))
        add(14, fch(3))
        add(15, ffin("q", 0, 2))
        add(16, ffin("q", 0, 3))
        # b1 granule work after all xt pieces have landed
        add(40, fp1("k", 1, 0))
        add(42, fp1("k", 1, 1))
        add(43, fch(4))
        add(44, ffin("k", 1, 0))
        add(45, ffin("k", 1, 1))
        add(46, fp1("k", 1, 2))
        add(47, fp1("k", 1, 3))
        add(48, fch(5))
        add(49, ffin("k", 1, 2))
        add(50, ffin("k", 1, 3))
        add(51, fp1("q", 1, 0))
        add(52, fp1("q", 1, 1))
        add(53, fch(6))
        add(54, ffin("q", 1, 0))
        add(55, ffin("q", 1, 1))
        add(56, fp1("q", 1, 2))
        add(57, fp1("q", 1, 3))
        add(58, fch(7))
        add(59, ffin("q", 1, 2))
        add(60, ffin("q", 1, 3))
        add(31, fvp(1, 0))
        add(32, fvb(1, 0, 0), fvb(1, 0, 1))
        add(33, fvb(1, 0, 2), fvb(1, 0, 3))
        add(34, fvp(1, 1))
        add(35, fvb(1, 1, 0), fvb(1, 1, 1))
        add(36, fvb(1, 1, 2), fvb(1, 1, 3))
        add(37, fvp(1, 2))
        add(38, fvb(1, 2, 0), fvb(1, 2, 1))
        add(39, fvb(1, 2, 2), fvb(1, 2, 3))
        add(61, fvp(1, 3))
        add(62, fvb(1, 3, 0), fvb(1, 3, 1))
        add(63, fvb(1, 3, 2), fvb(1, 3, 3))
        # outproj chunks: (b, ch) ready after it 16*(b*NJQ+ch)+16
        ostarts = {(0, 0): 18, (0, 1): 33, (0, 2): 63, (0, 3): 71,
                   (1, 0): 82, (1, 1): 98, (1, 2): 112, (1, 3): 128}
        for (b, chk), base in ostarts.items():
            for d in range(NCI):
                it = base + d
                add(min(it, 135), fout(b, chk, d))

        # ---- attention ----
        def qk_tile(b, q0, kt, slot):
            k0 = b * T + kt * 128
            nc.tensor.matmul(
                slot[0:128, 0:JQ],
                lhsT=kn_sb[0:64, k0 : k0 + 128],
                rhs=qn_sb[0:64, q0 : q0 + JQ],
                start=True, stop=True,
            )
            nc.tensor.matmul(
                slot[0:128, JQ : 2 * JQ],
                lhsT=kn_sb[64:128, k0 : k0 + 128],
                rhs=qn_sb[64:128, q0 : q0 + JQ],
                start=True, stop=True,
            )

        def y_normalize(b, jq):
            # copy y out of PSUM immediately so next jq's PV can start
            q0 = b * T + jq * JQ
            cpA = ycp.tile([65, JQ], dt.float32, tag="cp")
            cpB = ycp.tile([65, JQ], dt.float32, tag="cp")
            nc.vector.tensor_copy(cpA[:], yA_t[:])
            nc.vector.tensor_copy(cpB[:], yB_t[:])
            import os as _os
            if (b, jq) == (0, 0) and bool(int(_os.environ.get("BASS_ATTN_DEBUG", "0"))):
                for nm, ap in (("dbg_cpA", cpA[:]), ("dbg_cpB", cpB[:])):
                    extd = nc.dram_tensor(nm, list(ap.shape), ap.dtype,
                                          kind="ExternalOutput")
                    nc.sync.dma_start(out=extd.ap(), in_=ap)
            gA = stgp.tile([1, JQ], dt.float32, tag="g")
            gB = stgp.tile([1, JQ], dt.float32, tag="g")
            nc.sync.dma_start(out=gA[:], in_=cpA[64:65, :])
            nc.sync.dma_start(out=gB[:], in_=cpB[64:65, :])
            rA = stgp.tile([1, JQ], dt.float32, tag="r0")
            rB = stgp.tile([1, JQ], dt.float32, tag="r0")
            nc.vector.reciprocal_approx_fast(out=rA[:], in_=gA[:])
            nc.vector.reciprocal_approx_fast(out=rB[:], in_=gB[:])
            rbA = rbtp.tile([64, JQ], dt.float32, tag="rbB")
            rbB = rbtp.tile([64, JQ], dt.float32, tag="rbB")
            nc.gpsimd.partition_broadcast(rbA[:], rA[:])
            nc.gpsimd.partition_broadcast(rbB[:], rB[:])
            nc.vector.tensor_mul(yn_sb[0:64, q0 : q0 + JQ], cpA[0:64, :], rbA[:])
            ynB = ynbp.tile([64, JQ], dt.bfloat16, tag="ynB")
            nc.vector.tensor_mul(ynB[:], cpB[0:64, :], rbB[:])
            nc.sync.dma_start(out=yn_sb[64:128, q0 : q0 + JQ], in_=ynB[:])

        def attn_all():
            seq = [(b, jq) for b in range(B) for jq in range(NJQ)]
            slots = [slotA, slotB]
            qk_tile(0, 0, 0, slots[0])
            git = 0
            for si, (b, jq) in enumerate(seq):
                q0 = b * T + jq * JQ
                for kt in range(NKT):
                    cur = slots[git % 2]
                    nxt = slots[(git + 1) % 2]
                    se = sep.tile([128, 2 * JQ], dt.bfloat16, tag="se")
                    nc.scalar.activation(
                        out=se[:],
                        in_=cur[:],
                        func=AF.Exp,
                        bias=bias_sb[:, b * NKT + kt : b * NKT + kt + 1],
                        scale=EXPSCALE,
                    )
                    if kt + 1 < NKT:
                        qk_tile(b, q0, kt + 1, nxt)
                    elif si + 1 < len(seq):
                        nb, njq = seq[si + 1]
                        qk_tile(nb, nb * T + njq * JQ, 0, nxt)
                    import os as _os
                    if git in (0, 1, 4, 8, 12) and bool(
                        int(_os.environ.get("BASS_ATTN_DEBUG", "0"))
                    ):
                        e0 = nc.dram_tensor(f"dbg_se{git}", [128, 2 * JQ],
                                            dt.bfloat16, kind="ExternalOutput")
                        nc.sync.dma_start(out=e0.ap(), in_=se[:])
                    for th in fillers.get(git, ()):
                        th()
                    nc.tensor.matmul(
                        yA_t[:],
                        lhsT=vext[:, 0, b, kt, :],
                        rhs=se[:, 0:JQ],
                        start=(kt == 0), stop=(kt == NKT - 1),
                    )
                    nc.tensor.matmul(
                        yB_t[:],
                        lhsT=vext[:, 1, b, kt, :],
                        rhs=se[:, JQ : 2 * JQ],
                        start=(kt == 0), stop=(kt == NKT - 1),
                    )
                    git += 1
                y_normalize(b, jq)
            # post-loop fillers (tail outproj)
            for it in sorted(k for k in fillers if k >= git):
                for th in fillers[it]:
                    th()

        # ---- head ----
        qk_part1("q", 0, 0)
        qk_part1("k", 0, 0)
        chain(0)
        qk_finalize("q", 0, 0)
        qk_finalize("k", 0, 0)
        v_proj(0, 0)
        for jb in range(4):
            v_block(0, 0, jb)
        v_proj(0, 1)
        for jb in range(4):
            v_block(0, 1, jb)
        v_proj(0, 2)

        attn_all()

        if bool(int(os.environ.get("BASS_ATTN_DEBUG", "0"))):
            dbg_specs = [
                ("dbg_xt", xt_sb[:]),
                ("dbg_qn", qn_sb[:]),
                ("dbg_kn", kn_sb[:]),
                ("dbg_yn", yn_sb[:]),
                ("dbg_vext", vext[:].rearrange("p a b c d -> p (a b c d)")),
            ]
            for name, ap in dbg_specs:
                extd = nc.dram_tensor(name, list(ap.shape), ap.dtype,
                                      kind="ExternalOutput")
                nc.sync.dma_start(out=extd.ap(), in_=ap)

    nc.compile()
    _bacc_mod.get_activation_tables = _orig_tables
    return nc


def _get_nc():
    if "nc" not in _CACHE:
        _CACHE["nc"] = _build_bass()
    return _CACHE["nc"]


def _tile_major(a, width):
    return np.ascontiguousarray(
        a.reshape(NCI, 128, width).transpose(1, 0, 2).reshape(128, NCI * width)
    )


def _prep_in_maps(x, padding_mask, Wq, Wk, Wv, Wo):
    xf = np.ascontiguousarray(np.asarray(x, dtype=np.float32).reshape(BT, C))
    xt = _tile_major(np.ascontiguousarray(xf.T), BT).astype(BF16)
    xt = np.ascontiguousarray(
        xt.reshape(128, NCI, BT // GR, GR).transpose(0, 2, 1, 3).reshape(128, NCI * BT)
    )
    mb = np.where(
        np.asarray(padding_mask).reshape(BT), np.float32(0.0), np.float32(NEG)
    ).astype(np.float32)
    bias = np.ascontiguousarray(mb.reshape(B * NKT, 128).T)

    in_maps = []
    for i in range(NCORES):
        sl = slice(i * DPC, (i + 1) * DPC)
        in_maps.append(
            {
                "xt": xt,
                "wq": _tile_major(np.ascontiguousarray(Wq[sl, :].T), DPC).astype(BF16),
                "wk": _tile_major(np.ascontiguousarray(Wk[sl, :].T), DPC).astype(BF16),
                "wv": _tile_major(np.ascontiguousarray(Wv[sl, :].T), DPC).astype(BF16),
                "wo": np.ascontiguousarray(Wo[:, sl].T).astype(BF16),
                "bias": bias,
            }
        )
    return in_maps


def _assemble(results):
    total = np.zeros((NCI, 128, BT), dtype=np.float32)
    for r in results:
        total += (
            r["out"].reshape(128, NCI, BT).transpose(1, 0, 2).astype(np.float32)
        )
    return np.ascontiguousarray(total.reshape(C, BT).T).reshape(B, T, C)


def kernel(x, padding_mask, Wq, Wk, Wv, Wo):
    from concourse.bass_utils import run_bass_kernel_spmd

    nc = _get_nc()
    in_maps = _prep_in_maps(x, padding_mask, Wq, Wk, Wv, Wo)
    res = run_bass_kernel_spmd(nc, in_maps, core_ids=list(range(NCORES)))
    return _assemble(res.results)

